# revision 1
# baseline (speedup 1.0000x reference)
"""Trainium2 Bass kernel for BehaviorLemming (two fused stencil steps).

Sharding: data-parallel over batch. B=16 across 8 cores -> 2 batches/core.
Layout: H rows in partitions, (channel, W) in the free dim; input is
streamed per 4-channel group (1MB DMAs) so sets pipeline smoothly.
Per row-tile: DVE computes masks and the products P=a*w (Q=b*w on
GPSIMD); PE applies the +-1 row shifts as bit-exact fp32 matmuls with
shifted identity matrices, accumulating S_up@Q + S_dn@P in PSUM; the
"no move" case is patched with copy_predicated (uint8 m0 mask) after
ScalarE evacuates PSUM. Both steps run on-chip; the intermediate world
never touches HBM.

H tiling: 4 main sets of 124 output rows per batch (128 input rows incl.
2-row circular halo each side), plus ONE merged set handling the last 16
rows of BOTH batches (b0 at partitions 0..19, b1 at 32..51, block-
diagonal shift matrices).
"""

import numpy as np

_PQPOOL = [None]

import concourse.bacc as bacc
import concourse.mybir as mybir
import concourse.tile as tile
from concourse.bass_utils import run_bass_kernel_spmd

B, C, H, W = 16, 20, 512, 512
N_CORES = 8
B_PER_CORE = B // N_CORES
ELEM_ID = 3.0
F32 = mybir.dt.float32
U8 = mybir.dt.uint8
NCH = 4                 # channels per PSUM group (4 banks; bufs=2 -> 8)
NGRP = C // NCH
GP_Q = True             # Q products go to GPSIMD
MAIN_OUT = 124          # output rows per main set
MERGED_B1_OFF = 32      # partition offset of batch 1 rows in the merged set
MERGED_NP = 52
DMA_SHIFT_GROUPS = (1, 3)   # step-2 groups whose shifts ride DMA instead of PE


def _load_rows(nc, dst_tile, src_ap, row_start, n_rows, p0=0):
    """Load n_rows (mod H, split at wrap) of src [NCH,H,W] into dst
    partitions [p0, p0+n_rows), free dim = (c, w)."""
    s = row_start % H
    remaining = n_rows
    while remaining > 0:
        n = min(remaining, H - s)
        src = src_ap[:, s : s + n, :].rearrange("c h w -> h c w")
        nc.sync.dma_start(out=dst_tile[p0 : p0 + n, :].rearrange(
            "h (c w) -> h c w", c=NCH), in_=src)
        p0 += n
        s = (s + n) % H
        remaining -= n


def _build_masks(nc, pool, pmain, su, sd, world_t, np_, shift_w):
    """Masks for one step. world_t's free dim starts with ch0 (elem ids)
    then ch1 (density). Returns (a_f32, b_f32, m0_u8) SBUF tiles."""
    al = mybir.AluOpType
    e = world_t[0:np_, 0:W]
    d = world_t[0:np_, W : 2 * W]

    # dR = roll(d, shift_w) along the free (W) axis
    dR = pool.tile([np_, W], F32, tag="dR")
    if shift_w == 1:
        nc.scalar.copy(dR[:, 1:W], d[:, 0 : W - 1])
        nc.scalar.copy(dR[:, 0:1], d[:, W - 1 : W])
    else:
        nc.scalar.copy(dR[:, 0 : W - 1], d[:, 1:W])
        nc.scalar.copy(dR[:, W - 1 : W], d[:, 0:1])

    # mask shift matmuls share one pmain slot: dA | dAR | b
    mp = pmain.tile([np_, NCH * W], F32, tag="ps")
    dA = mp[:, 0:W]
    dAR = mp[:, W : 2 * W]
    bp = mp[:, 2 * W : 3 * W]
    nc.tensor.matmul(out=dA, lhsT=su, rhs=d, start=True, stop=True)
    nc.tensor.matmul(out=dAR, lhsT=su, rhs=dR[:], start=True, stop=True)

    c1 = pool.tile([np_, W], F32, tag="c1")
    c2 = pool.tile([np_, W], F32, tag="c2")
    c3 = pool.tile([np_, W], F32, tag="c3")
    nc.vector.tensor_tensor(out=c1[:], in0=dR[:], in1=d, op=al.is_ge)
    nc.vector.tensor_tensor(out=c2[:], in0=dA, in1=d, op=al.is_lt)
    nc.vector.tensor_tensor(out=c3[:], in0=dAR, in1=d, op=al.is_lt)
    e3c3 = pool.tile([np_, W], F32, tag="e3")
    nc.vector.scalar_tensor_tensor(out=e3c3[:], in0=e, scalar=ELEM_ID,
                                   in1=c3[:], op0=al.is_equal,
                                   op1=al.logical_and)
    c12 = pool.tile([np_, W], F32, tag="c12")
    nc.vector.tensor_tensor(out=c12[:], in0=c1[:], in1=c2[:],
                            op=al.logical_and)
    a = pool.tile([np_, W], F32, tag="a")
    nc.vector.tensor_tensor(out=a[:], in0=c12[:], in1=e3c3[:],
                            op=al.logical_and)

    # b[p] = a[p+1]; evacuate to SBUF so the psum slot frees quickly
    nc.tensor.matmul(out=bp, lhsT=sd, rhs=a[:], start=True, stop=True)
    b = pool.tile([np_, W], F32, tag="b")
    nc.scalar.copy(b[:], bp)

    # m0 = (a | b) == 0, as uint8 for copy_predicated
    r = pool.tile([np_, W], F32, tag="r")
    nc.vector.tensor_tensor(out=r[:], in0=a[:], in1=b[:], op=al.logical_or)
    m0 = pool.tile([np_, W], U8, tag="m0")
    nc.vector.tensor_scalar(out=m0[:], in0=r[:], scalar1=0.0, scalar2=None,
                            op0=al.is_equal)
    return a, b, m0


def _step_combine(nc, pool, pmain, su, sd, src_g, a, b, m0, np_, dst_g,
                  shift_via_dma=False, q_on_dve=False):
    """One stencil step for one NCH-channel group:
    dst = m0 ? src : (S_up@(b*src) + S_dn@(a*src)).

    shift_via_dma: apply the row shifts with SBUF->SBUF accumulating DMAs
    instead of PE matmuls (dst rows 0 / np_-1 end up garbage; only legal
    when those rows are never consumed, i.e. step-2 output tiles)."""
    al = mybir.AluOpType
    fd = NCH * W
    src_v = src_g.rearrange("p (c w) -> p c w", c=NCH)
    a_b = a[:].unsqueeze(1).broadcast_to([np_, NCH, W])
    b_b = b[:].unsqueeze(1).broadcast_to([np_, NCH, W])
    m0_b = m0[:].unsqueeze(1).broadcast_to([np_, NCH, W])

    P = _PQPOOL[0].tile([np_, fd], F32, tag="P")
    Q = _PQPOOL[0].tile([np_, fd], F32, tag="Q")
    nc.vector.tensor_tensor(out=P[:].rearrange("p (c w) -> p c w", c=NCH),
                            in0=src_v, in1=a_b, op=al.mult)
    qeng = nc.vector if (q_on_dve or not GP_Q) else nc.gpsimd
    qeng.tensor_tensor(out=Q[:].rearrange("p (c w) -> p c w", c=NCH),
                       in0=src_v, in1=b_b, op=al.mult)

    if shift_via_dma:
        # dst[p] = Q[p-1]; then dst[p] += P[p+1]
        nc.gpsimd.dma_start(out=dst_g.tensor[1:np_, 0:fd],
                            in_=Q[0 : np_ - 1, :])
        nc.gpsimd.dma_start(out=dst_g.tensor[0 : np_ - 1, 0:fd],
                            in_=P[1:np_, :], accum_op=al.add)
    else:
        ps = pmain.tile([np_, fd], F32, tag="ps")
        for c in range(NCH):
            nc.tensor.matmul(out=ps[:, c * W : (c + 1) * W], lhsT=su,
                             rhs=Q[:, c * W : (c + 1) * W],
                             start=True, stop=False)
        for c in range(NCH):
            nc.tensor.matmul(out=ps[:, c * W : (c + 1) * W], lhsT=sd,
                             rhs=P[:, c * W : (c + 1) * W],
                             start=False, stop=True)
        nc.scalar.copy(dst_g, ps[:])
    nc.vector.copy_predicated(dst_g.rearrange("p (c w) -> p c w", c=NCH),
                              m0_b, src_v)


def _new_set_state(nc, pools, sd):
    """Allocate w1, load group 0 and build step-1 masks for a set."""
    wpool, bigpool, opool, pool, pmain = pools
    sup, sdn, np_, load_group, _sg = sd
    g0 = wpool.tile([128, NCH * W], F32, tag="w0g")
    load_group(g0, 0)
    masks1 = _build_masks(nc, pool, pmain, sup, sdn, g0, np_, 1)
    w1 = bigpool.tile([128, C * W], F32, tag="w1")
    return {"sd": sd, "g0": g0, "masks1": masks1, "w1": w1, "masks2": None}


def _step1_group(nc, pools, st, g):
    """Emit step-1 for one channel group of a set."""
    wpool, bigpool, opool, pool, pmain = pools
    sup, sdn, np_, load_group, _sg = st["sd"]
    a1, b1, m01 = st["masks1"]
    if g == 0:
        t = st["g0"]
    else:
        t = wpool.tile([128, NCH * W], F32, tag="w0g")
        load_group(t, g)
    dst = st["w1"][0:np_, g * NCH * W : (g + 1) * NCH * W]
    _step_combine(nc, pool, pmain, sup, sdn, t[0:np_, :],
                  a1, b1, m01, np_, dst, q_on_dve=(g == 0))
    if g == 0:
        # step-2 masks only need w1 ch0/ch1: emit now so the mask DVE
        # chain overlaps other groups' PE work
        st["masks2"] = _build_masks(nc, pool, pmain, sup, sdn,
                                    st["w1"], np_, -1)


def _step2_group(nc, pools, st, g):
    """Emit step-2 + store for one channel group of a set."""
    wpool, bigpool, opool, pool, pmain = pools
    sup, sdn, np_, _lg, store_group = st["sd"]
    a2, b2, m02 = st["masks2"]
    src = st["w1"][0:np_, g * NCH * W : (g + 1) * NCH * W]
    og = opool.tile([128, NCH * W], F32, tag="og")
    _step_combine(nc, pool, pmain, sup, sdn, src,
                  a2, b2, m02, np_, og[0:np_, :],
                  shift_via_dma=(g in DMA_SHIFT_GROUPS),
                  q_on_dve=(g == 0))
    store_group(og, g)


def build_kernel():
    nc = bacc.Bacc("TRN2", target_bir_lowering=False, debug=False,
                   num_devices=N_CORES)
    wd = nc.dram_tensor("world", [B_PER_CORE, C, H, W], F32,
                        kind="ExternalInput").ap()
    su_d = nc.dram_tensor("s_up", [128, 128], F32, kind="ExternalInput").ap()
    sd_d = nc.dram_tensor("s_dn", [128, 128], F32, kind="ExternalInput").ap()
    sum_d = nc.dram_tensor("s_up_m", [MERGED_NP, MERGED_NP], F32,
                           kind="ExternalInput").ap()
    sdm_d = nc.dram_tensor("s_dn_m", [MERGED_NP, MERGED_NP], F32,
                           kind="ExternalInput").ap()
    od = nc.dram_tensor("out", [B_PER_CORE, C, H, W], F32,
                        kind="ExternalOutput").ap()

    with tile.TileContext(nc) as tc:
        with (
            tc.tile_pool(name="const", bufs=1) as cpool,
            tc.tile_pool(name="wpool", bufs=3) as wpool,
            tc.tile_pool(name="big", bufs=2) as bigpool,
            tc.tile_pool(name="opool", bufs=2) as opool,
            tc.tile_pool(name="small", bufs=2) as pool,
            tc.tile_pool(name="pq", bufs=3) as pqpool,
            tc.tile_pool(name="pmain", bufs=2, space="PSUM") as pmain,
        ):
            st_up = cpool.tile([128, 128], F32)
            st_dn = cpool.tile([128, 128], F32)
            st_up_m = cpool.tile([MERGED_NP, MERGED_NP], F32)
            st_dn_m = cpool.tile([MERGED_NP, MERGED_NP], F32)
            nc.sync.dma_start(out=st_up[:], in_=su_d)
            nc.sync.dma_start(out=st_dn[:], in_=sd_d)
            nc.sync.dma_start(out=st_up_m[:], in_=sum_d)
            nc.sync.dma_start(out=st_dn_m[:], in_=sdm_d)

            _PQPOOL[0] = pqpool
            pools = (wpool, bigpool, opool, pool, pmain)

            def make_main_set(bi, si):
                r_out = si * MAIN_OUT

                def load_group(t, g):
                    src = wd[bi, g * NCH : (g + 1) * NCH]
                    _load_rows(nc, t, src, r_out - 2, 128)

                def store_group(og, g):
                    dst = od[bi, g * NCH : (g + 1) * NCH,
                             r_out : r_out + MAIN_OUT, :]
                    nc.sync.dma_start(
                        out=dst.rearrange("c h w -> h c w"),
                        in_=og[2 : 2 + MAIN_OUT, :].rearrange(
                            "h (c w) -> h c w", c=NCH))

                return (st_up[:], st_dn[:], 128, load_group, store_group)

            def make_merged_set():
                r_out = 4 * MAIN_OUT      # 496
                n_out = H - r_out         # 16

                def load_group(t, g):
                    # zero first (aligned range) so gap partitions between
                    # the batch blocks can't feed NaN garbage into the PE
                    nc.gpsimd.memset(t[0:64, :], 0.0)
                    for bi, p0 in ((0, 0), (1, MERGED_B1_OFF)):
                        src = wd[bi, g * NCH : (g + 1) * NCH]
                        _load_rows(nc, t, src, r_out - 2, n_out + 4, p0=p0)

                def store_group(og, g):
                    for bi, p0 in ((0, 2), (1, MERGED_B1_OFF + 2)):
                        dst = od[bi, g * NCH : (g + 1) * NCH,
                                 r_out : r_out + n_out, :]
                        nc.sync.dma_start(
                            out=dst.rearrange("c h w -> h c w"),
                            in_=og[p0 : p0 + n_out, :].rearrange(
                                "h (c w) -> h c w", c=NCH))

                return (st_up_m[:], st_dn_m[:], MERGED_NP, load_group,
                        store_group)

            sets = [make_main_set(bi, si)
                    for bi in range(B_PER_CORE) for si in range(4)]
            sets.append(make_merged_set())

            # software-pipelined emission: the NEXT set's g0 load + step-1
            # masks are emitted before the CURRENT set's step-2 groups
            st = _new_set_state(nc, pools, sets[0])
            _step1_group(nc, pools, st, 0)
            for i in range(len(sets)):
                for g in range(1, NGRP):
                    _step1_group(nc, pools, st, g)
                st_next = (_new_set_state(nc, pools, sets[i + 1])
                           if i + 1 < len(sets) else None)
                for g in range(NGRP):
                    _step2_group(nc, pools, st, g)
                    if g == 2 and st_next is not None:
                        # inject the next set's first step-1 group so the
                        # PE stream stays dense through the step-2 tail
                        _step1_group(nc, pools, st_next, 0)
                st = st_next

    nc.compile()
    return nc


def _shift_mats():
    s_up = np.zeros((128, 128), np.float32)  # out[m] = in[m-1]
    s_dn = np.zeros((128, 128), np.float32)  # out[m] = in[m+1]
    for m in range(128):
        if m >= 1:
            s_up[m - 1, m] = 1.0
        if m <= 126:
            s_dn[m + 1, m] = 1.0
    s_up_m = np.zeros((MERGED_NP, MERGED_NP), np.float32)
    s_dn_m = np.zeros((MERGED_NP, MERGED_NP), np.float32)
    for base in (0, MERGED_B1_OFF):
        for m in range(20):
            if m >= 1:
                s_up_m[base + m - 1, base + m] = 1.0
            if m <= 18:
                s_dn_m[base + m + 1, base + m] = 1.0
    return s_up, s_dn, s_up_m, s_dn_m


_NC_CACHE = {}


def kernel(world, rand_movement=None, rand_interact=None, rand_element=None,
           **_ignored):
    world = np.ascontiguousarray(world, dtype=np.float32)
    assert world.shape == (B, C, H, W), world.shape
    if "nc" not in _NC_CACHE:
        _NC_CACHE["nc"] = build_kernel()
    nc = _NC_CACHE["nc"]
    s_up, s_dn, s_up_m, s_dn_m = _shift_mats()
    in_maps = []
    for core in range(N_CORES):
        shard = world[core * B_PER_CORE : (core + 1) * B_PER_CORE]
        in_maps.append({"world": np.ascontiguousarray(shard),
                        "s_up": s_up, "s_dn": s_dn,
                        "s_up_m": s_up_m, "s_dn_m": s_dn_m})
    res = run_bass_kernel_spmd(nc, in_maps, list(range(N_CORES)),
                               trace=_NC_CACHE.get("trace", False))
    _NC_CACHE["last_result"] = res
    out = np.concatenate([r["out"] for r in res.results], axis=0)
    return out.astype(np.float32)


if __name__ == "__main__":
    rng = np.random.default_rng(0)
    w = rng.standard_normal((B, C, H, W)).astype(np.float32)
    w[:, 0] = rng.integers(0, 10, (B, 1, H, W)).astype(np.float32)[:, 0]
    out = kernel(w)
    print("ran:", out.shape, out.dtype)



# revision 3
# speedup vs baseline: 1.0006x; 1.0006x over previous
"""Trainium2 Bass kernel for BehaviorLemming, v3.

Two fused stencil steps, data-parallel over batch (B=16 / 8 cores).
Layout: H rows in partitions, (channel, W) in free dim, 5 groups of 4ch.

v3 vs baseline:
- World movement in fp16: products P=a*w, Q=b*w, R=m0*w as fp16 tiles;
  row shifts as fp16 matmuls (1 cyc/row vs fp32's 4). The stay term R
  rides a third identity-matmul chain into PSUM, so the copy_predicated
  pass disappears; PSUM = su@Q + sd@P + I@R is the complete output.
- Exact fp32 side-path for step-1 density (ch1) only: step-2 mask
  comparisons must see bit-exact step-1 densities. Final outputs
  tolerate fp16 rounding (gate 2e-2, fp16 gives ~5e-4).
- Mask row-shifts (b = roll(a,-1), dA2 = roll(d2,+1)) via SBUF->SBUF
  DMA, density-above (dA1) loaded straight from HBM at a row offset:
  no mask matmuls on PE.
- Stores in fp16 (host converts): ~half the store traffic.
- Elementwise spread: P/R + small stts on DVE (fp16 2x), Q + compares
  on Pool, conversions + PSUM evacuations on Act.
"""

import numpy as np

import concourse.bacc as bacc
import concourse.mybir as mybir
import concourse.tile as tile
from concourse.bass_utils import run_bass_kernel_spmd

B, C, H, W = 16, 20, 512, 512
N_CORES = 8
B_PER_CORE = B // N_CORES
ELEM_ID = 3.0
F32 = mybir.dt.float32
F16 = mybir.dt.float16
NCH = 4
NGRP = C // NCH
MAIN_OUT = 124
M_B1 = 32              # partition offset of batch-1 block in the merged set
M_NP = 52

# knob per (step, group): where the Q product runs.
# "pool" = all 4ch on Pool, "split" = 2ch Pool + 2ch DVE, "dve" = all DVE
QMODE = {}
for _s in (1, 2):
    for _g in range(5):
        QMODE[(_s, _g)] = "split"
QMODE[(1, 4)] = "pool"
QMODE[(2, 4)] = "pool"
QMODE[(1, 0)] = "pool"
# which groups' conv (fp32->fp16 world copy) run on Act (rest DVE)
CONV_ACT = {0, 1, 2, 3, 4}


def _load_rows(nc, dst_tile, src_ap, row_start, n_rows, p0=0, nch=NCH):
    """Load n_rows (mod H, split at wrap) of src [nch,H,W] into dst
    partitions [p0, p0+n_rows), free dim = (c, w)."""
    s = row_start % H
    remaining = n_rows
    while remaining > 0:
        n = min(remaining, H - s)
        src = src_ap[:, s : s + n, :].rearrange("c h w -> h c w")
        nc.sync.dma_start(out=dst_tile[p0 : p0 + n, :].rearrange(
            "h (c w) -> h c w", c=nch), in_=src)
        p0 += n
        s = (s + n) % H
        remaining -= n


def _cmp_rolled(nc, al, out, rolled_src, base, shift_w, op):
    """out = op(roll(rolled_src, shift_w, W), base), via shifted free-axis
    APs: no materialized roll. Two pieces (bulk + 1-col wrap)."""
    if shift_w == 1:
        nc.vector.tensor_tensor(out=out[:, 1:W], in0=rolled_src[:, 0 : W - 1],
                                in1=base[:, 1:W], op=op)
        nc.vector.tensor_tensor(out=out[:, 0:1], in0=rolled_src[:, W - 1 : W],
                                in1=base[:, 0:1], op=op)
    else:
        nc.vector.tensor_tensor(out=out[:, 0 : W - 1], in0=rolled_src[:, 1:W],
                                in1=base[:, 0 : W - 1], op=op)
        nc.vector.tensor_tensor(out=out[:, W - 1 : W], in0=rolled_src[:, 0:1],
                                in1=base[:, W - 1 : W], op=op)


class SetCtx:
    """Per-set emission state."""

    def __init__(self, sd):
        self.sd = sd
        self.wb = None        # [np,10240] f16 world
        self.g0 = None        # [np,2048] f32 (ch0..3) for masks + exact
        self.dA1 = None       # [np,512] f32 density rolled +1 (HBM load)
        self.w1b = None       # [np,10240] f16 step-1 world
        self.w1d = None       # [np,512] f32 exact step-1 density
        self.m1 = None        # (a16, b16, m016) step-1
        self.m2 = None


def build_kernel():
    nc = bacc.Bacc("TRN2", target_bir_lowering=False, debug=False,
                   num_devices=N_CORES)
    wd = nc.dram_tensor("world", [B_PER_CORE, C, H, W], F32,
                        kind="ExternalInput").ap()
    su32_d = nc.dram_tensor("su32", [128, 128], F32, kind="ExternalInput").ap()
    sd32_d = nc.dram_tensor("sd32", [128, 128], F32, kind="ExternalInput").ap()
    su16_d = nc.dram_tensor("su16", [128, 128], F16, kind="ExternalInput").ap()
    sd16_d = nc.dram_tensor("sd16", [128, 128], F16, kind="ExternalInput").ap()
    i16_d = nc.dram_tensor("i16", [128, 128], F16, kind="ExternalInput").ap()
    sum32_d = nc.dram_tensor("sum32", [M_NP, M_NP], F32,
                             kind="ExternalInput").ap()
    sdm32_d = nc.dram_tensor("sdm32", [M_NP, M_NP], F32,
                             kind="ExternalInput").ap()
    sum16_d = nc.dram_tensor("sum16", [M_NP, M_NP], F16,
                             kind="ExternalInput").ap()
    sdm16_d = nc.dram_tensor("sdm16", [M_NP, M_NP], F16,
                             kind="ExternalInput").ap()
    od = nc.dram_tensor("out16", [B_PER_CORE, C, H, W], F16,
                        kind="ExternalOutput").ap()

    al = mybir.AluOpType

    with tile.TileContext(nc) as tc:
        with (
            tc.tile_pool(name="const", bufs=1) as cpool,
            tc.tile_pool(name="stg", bufs=2) as stgpool,
            tc.tile_pool(name="g0p", bufs=2) as g0pool,
            tc.tile_pool(name="wbp", bufs=2) as wbpool,
            tc.tile_pool(name="w1p", bufs=2) as w1pool,
            tc.tile_pool(name="mkp", bufs=2) as mkpool,
            tc.tile_pool(name="pqr", bufs=3) as pqrpool,
            tc.tile_pool(name="ogp", bufs=2) as ogpool,
            tc.tile_pool(name="pmain", bufs=4, space="PSUM") as pmain,
        ):
            su32 = cpool.tile([128, 128], F32)
            sd32 = cpool.tile([128, 128], F32)
            su16 = cpool.tile([128, 128], F16)
            sd16 = cpool.tile([128, 128], F16)
            i16 = cpool.tile([128, 128], F16)
            sum32 = cpool.tile([M_NP, M_NP], F32)
            sdm32 = cpool.tile([M_NP, M_NP], F32)
            sum16 = cpool.tile([M_NP, M_NP], F16)
            sdm16 = cpool.tile([M_NP, M_NP], F16)
            for t, d in ((su32, su32_d), (sd32, sd32_d), (su16, su16_d),
                         (sd16, sd16_d), (i16, i16_d), (sum32, sum32_d),
                         (sdm32, sdm32_d), (sum16, sum16_d), (sdm16, sdm16_d)):
                nc.sync.dma_start(out=t[:], in_=d)
            ones16 = cpool.tile([128, W], F16)
            nc.gpsimd.memset(ones16[:], 1.0)

            def masks_rolls(st, step):
                """Phase 1: dA2 = roll(d2,+1,H) via fp32 matmul (step 2)."""
                sd = st.sd
                np_ = sd["np"]
                if step == 1:
                    d = st.g0[0:np_, W : 2 * W]
                    dA = st.dA1[0:np_, :]
                    return {"d": d, "dA": dA}
                d = st.w1d[0:np_, :]
                psx = pmain.tile([np_, 2 * W], F32, tag="ps")
                nc.tensor.matmul(out=psx[:, 0:W], lhsT=sd["su32"], rhs=d,
                                 start=True, stop=True)
                dAt = mkpool.tile([np_, W], F32, tag="dA2")
                nc.scalar.copy(dAt[:], psx[:, 0:W])
                return {"d": d, "dA": dAt[:], "psx": psx}

            def masks_cmps(st, step, mc):
                """Phase 2: density comparisons via shifted free-axis APs."""
                np_ = st.sd["np"]
                shift_w = 1 if step == 1 else -1
                d, dA = mc["d"], mc["dA"]
                c1 = mkpool.tile([np_, W], F16, tag="c1")
                c2 = mkpool.tile([np_, W], F16, tag="c2")
                c3 = mkpool.tile([np_, W], F16, tag="c3")
                _cmp_rolled(nc, al, c1, d, d, shift_w, al.is_ge)
                nc.vector.tensor_tensor(out=c2[:], in0=dA, in1=d, op=al.is_lt)
                _cmp_rolled(nc, al, c3, dA, d, shift_w, al.is_lt)
                mc.update(c1=c1, c2=c2, c3=c3)

            def masks_chain(st, step, mc):
                """Phase 3: AND-tree -> a16; b16 = roll(a,-1,H) via matmul."""
                sd = st.sd
                np_ = sd["np"]
                e = st.g0[0:np_, 0:W] if step == 1 else st.w1b[0:np_, 0:W]
                e3c3 = mkpool.tile([np_, W], F16, tag="e3")
                nc.vector.scalar_tensor_tensor(out=e3c3[:], in0=e,
                                               scalar=ELEM_ID,
                                               in1=mc["c3"][:],
                                               op0=al.is_equal,
                                               op1=al.logical_and)
                c12 = mkpool.tile([np_, W], F16, tag="c12")
                nc.vector.tensor_tensor(out=c12[:], in0=mc["c1"][:],
                                        in1=mc["c2"][:], op=al.logical_and)
                mp = mkpool.tile([np_, 2 * W], F16, tag="mp")
                a16 = mp[:, 0:W]
                nc.vector.tensor_tensor(out=a16, in0=c12[:], in1=e3c3[:],
                                        op=al.logical_and)
                # b16[m] = a16[m+1]; sd16 zeroes the boundary rows natively
                psx = mc.get("psx")
                if psx is None:
                    psx = pmain.tile([np_, 2 * W], F32, tag="ps")
                    mc["psx"] = psx
                nc.tensor.matmul(out=psx[:, W : 2 * W], lhsT=sd["sd16"],
                                 rhs=a16, start=True, stop=True)
                b16 = mkpool.tile([np_, W], F16, tag="b16")
                nc.scalar.copy(b16[:], psx[:, W : 2 * W])
                mc.update(a16=a16, b16=b16, mp=mp)

            def masks_fin(st, step, mc):
                """Phase 4: r16/m016 (after the b16 DMA has had time)."""
                np_ = st.sd["np"]
                a16, b16 = mc["a16"], mc["b16"]
                r16 = mkpool.tile([np_, W], F16, tag="r16")
                nc.vector.tensor_tensor(out=r16[:], in0=a16, in1=b16[:],
                                        op=al.logical_or)
                m016 = mc["mp"][:, W : 2 * W]
                # r < 1 == (r == 0) for 0/1 masks; all-f16 operands -> 2x DVE
                nc.vector.tensor_tensor(out=m016, in0=r16[:],
                                        in1=ones16[0:np_, :], op=al.is_lt)
                if step == 1:
                    st.m1 = (mc["mp"], b16)
                else:
                    st.m2 = (mc["mp"], b16)

            def exact1(st, mc):
                """Exact fp32 density path (feeds step-2 comparisons)."""
                sd = st.sd
                np_ = sd["np"]
                d = st.g0[0:np_, W : 2 * W]
                mp, b16 = st.m1
                a16 = mp[:, 0:W]
                m016 = mp[:, W : 2 * W]
                P0 = mkpool.tile([np_, W], F32, tag="P0")
                Q0 = mkpool.tile([np_, W], F32, tag="Q0")
                R0 = mkpool.tile([np_, W], F32, tag="R0")
                nc.vector.tensor_tensor(out=P0[:], in0=a16, in1=d,
                                        op=al.mult)
                nc.vector.tensor_tensor(out=Q0[:], in0=b16[:], in1=d,
                                        op=al.mult)
                nc.vector.tensor_tensor(out=R0[:], in0=m016, in1=d,
                                        op=al.mult)
                psd = pmain.tile([np_, 2 * W], F32, tag="ps")
                nc.tensor.matmul(out=psd[:, 0:W], lhsT=sd["su32"],
                                 rhs=Q0[:], start=True, stop=False)
                nc.tensor.matmul(out=psd[:, 0:W], lhsT=sd["sd32"],
                                 rhs=P0[:], start=False, stop=True)
                w1d = w1pool.tile([np_, W], F32, tag="w1d")
                nc.vector.tensor_tensor(out=w1d[:], in0=psd[:, 0:W],
                                        in1=R0[:], op=al.add)
                st.w1d = w1d

            def emit_group(st, step, g):
                """One fp16 stencil group-step: products, matmuls, evac."""
                sd = st.sd
                np_ = sd["np"]
                mp, b16 = st.m1 if step == 1 else st.m2
                src = (st.wb if step == 1 else st.w1b)[
                    0:np_, g * NCH * W : (g + 1) * NCH * W]
                src_v = src.rearrange("p (c w) -> p c w", c=NCH)
                b_b = b16[:].unsqueeze(1).broadcast_to([np_, NCH, W])
                fd = NCH * W
                # P and R as ONE double-wide DVE op: out [np, 2, NCH, W],
                # masks [a16 | m016] broadcast over channels, src broadcast
                # over the P/R axis.
                PR = pqrpool.tile([np_, 2 * fd], F16, tag="PR")
                mode = QMODE[(step, g)]
                if mode == "poolR":
                    a_b = mp[:, 0:W].unsqueeze(1).broadcast_to(
                        [np_, NCH, W])
                    m_b = mp[:, W : 2 * W].unsqueeze(1).broadcast_to(
                        [np_, NCH, W])
                    nc.vector.tensor_tensor(
                        out=PR[:, 0:fd].rearrange("p (c w) -> p c w", c=NCH),
                        in0=a_b, in1=src_v, op=al.mult)
                    nc.gpsimd.tensor_tensor(
                        out=PR[:, fd : 2 * fd].rearrange(
                            "p (c w) -> p c w", c=NCH),
                        in0=m_b, in1=src_v, op=al.mult)
                else:
                    PRv = PR[:].rearrange("p (k c w) -> p k c w", k=2, c=NCH)
                    mp_b = mp.rearrange("p (k w) -> p k w", k=2).unsqueeze(
                        2).broadcast_to([np_, 2, NCH, W])
                    src_b = src_v.unsqueeze(1).broadcast_to([np_, 2, NCH, W])
                    nc.vector.tensor_tensor(out=PRv, in0=mp_b, in1=src_b,
                                            op=al.mult)
                P = PR[:, 0:fd]
                R = PR[:, fd : 2 * fd]
                Q = pqrpool.tile([np_, fd], F16, tag="Q")
                Qv = Q[:].rearrange("p (c w) -> p c w", c=NCH)
                if mode == "split":
                    h = NCH // 2
                    nc.gpsimd.tensor_tensor(
                        out=Qv[:, 0:h], in0=b_b[:, 0:h], in1=src_v[:, 0:h],
                        op=al.mult)
                    nc.vector.tensor_tensor(
                        out=Qv[:, h:NCH], in0=b_b[:, h:NCH],
                        in1=src_v[:, h:NCH], op=al.mult)
                else:
                    qeng = nc.vector if mode == "dve" else nc.gpsimd
                    qeng.tensor_tensor(out=Qv, in0=b_b, in1=src_v, op=al.mult)
                if step == 1:
                    og = None
                    dst = st.w1b[0:np_, g * fd : (g + 1) * fd]
                else:
                    og = ogpool.tile([np_, fd], F16, tag="og")
                    dst = og[0:np_, :]
                hw = 2 * W
                for h in range(2):
                    ps = pmain.tile([np_, hw], F32, tag="ps")
                    for c in (2 * h, 2 * h + 1):
                        r = slice((c - 2 * h) * W, (c - 2 * h + 1) * W)
                        nc.tensor.matmul(out=ps[:, r], lhsT=sd["su16"],
                                         rhs=Q[:, c * W : (c + 1) * W],
                                         start=True, stop=False)
                        nc.tensor.matmul(out=ps[:, r], lhsT=sd["sd16"],
                                         rhs=P[:, c * W : (c + 1) * W],
                                         start=False, stop=False)
                        nc.tensor.matmul(out=ps[:, r], lhsT=sd["i16"],
                                         rhs=R[:, c * W : (c + 1) * W],
                                         start=False, stop=True)
                    nc.scalar.copy(dst[:, h * hw : (h + 1) * hw], ps[:])
                if og is not None:
                    sd["store"](og, g)

            def prep_A(sd):
                """Next-set loads for g0 + dA1, conv g0."""
                st = SetCtx(sd)
                np_ = sd["np"]
                st.g0 = g0pool.tile([128, NCH * W], F32, tag="g0")
                sd["load"](st.g0, 0)
                st.dA1 = g0pool.tile([128, W], F32, tag="dA1")
                sd["load_dA1"](st.dA1)
                st.wb = wbpool.tile([128, C * W], F16, tag="wb")
                nc.scalar.copy(st.wb[0:np_, 0 : NCH * W], st.g0[0:np_, :])
                return st

            def prep_B(st):
                """Remaining group loads + conversions."""
                np_ = st.sd["np"]
                for g in range(1, NGRP):
                    stg = stgpool.tile([128, NCH * W], F32, tag="stg")
                    st.sd["load"](stg, g)
                    dst = st.wb[0:np_, g * NCH * W : (g + 1) * NCH * W]
                    if g in CONV_ACT:
                        nc.scalar.copy(dst, stg[0:np_, :])
                    else:
                        nc.vector.tensor_copy(dst, stg[0:np_, :])

            def make_main_set(bi, si):
                r_out = si * MAIN_OUT

                def load(t, g):
                    _load_rows(nc, t, wd[bi, g * NCH : (g + 1) * NCH],
                               r_out - 2, 128)

                def load_dA1(t):
                    _load_rows(nc, t, wd[bi, 1:2], r_out - 3, 128, nch=1)

                def store(og, g):
                    dst = od[bi, g * NCH : (g + 1) * NCH,
                             r_out : r_out + MAIN_OUT, :]
                    nc.scalar.dma_start(
                        out=dst.rearrange("c h w -> h c w"),
                        in_=og[2 : 2 + MAIN_OUT, :].rearrange(
                            "h (c w) -> h c w", c=NCH))

                return {"np": 128, "su32": su32[:], "sd32": sd32[:],
                        "su16": su16[:], "sd16": sd16[:],
                        "i16": i16[:], "blocks": [(0, 128)],
                        "load": load, "load_dA1": load_dA1, "store": store}

            def make_merged_set():
                r_out = 4 * MAIN_OUT
                n_out = H - r_out        # 16
                blocks = [(0, n_out + 4), (M_B1, n_out + 4)]

                def load(t, g):
                    nc.gpsimd.memset(t[0:64, :], 0.0)
                    for bi, p0 in ((0, 0), (1, M_B1)):
                        _load_rows(nc, t, wd[bi, g * NCH : (g + 1) * NCH],
                                   r_out - 2, n_out + 4, p0=p0)

                def load_dA1(t):
                    nc.gpsimd.memset(t[0:64, :], 0.0)
                    for bi, p0 in ((0, 0), (1, M_B1)):
                        _load_rows(nc, t, wd[bi, 1:2], r_out - 3, n_out + 4,
                                   p0=p0, nch=1)

                def store(og, g):
                    for bi, p0 in ((0, 2), (1, M_B1 + 2)):
                        dst = od[bi, g * NCH : (g + 1) * NCH,
                                 r_out : r_out + n_out, :]
                        nc.scalar.dma_start(
                            out=dst.rearrange("c h w -> h c w"),
                            in_=og[p0 : p0 + n_out, :].rearrange(
                                "h (c w) -> h c w", c=NCH))

                return {"np": M_NP, "su32": sum32[:], "sd32": sdm32[:],
                        "su16": sum16[:], "sd16": sdm16[:],
                        "i16": i16[0:M_NP, 0:M_NP], "blocks": blocks,
                        "load": load, "load_dA1": load_dA1, "store": store}

            sets = [make_main_set(bi, si)
                    for bi in range(B_PER_CORE) for si in range(4)]
            sets.append(make_merged_set())

            # deep software pipeline. Per-engine queues run in emission
            # order, so long-latency chains (partition-shift DMAs, evac-
            # dependent compares) are split into phases and interleaved
            # between bulk product groups that hide their latency.
            st = prep_A(sets[0])
            prep_B(st)
            mc1 = masks_rolls(st, 1)
            masks_cmps(st, 1, mc1)
            masks_chain(st, 1, mc1)
            masks_fin(st, 1, mc1)
            exact1(st, mc1)
            st.w1b = w1pool.tile([128, C * W], F16, tag="w1b")
            for i in range(len(sets)):
                # phase B: step 1 with step-2 mask phases interleaved
                emit_group(st, 1, 0)
                mc2 = masks_rolls(st, 2)
                masks_cmps(st, 2, mc2)
                emit_group(st, 1, 1)
                masks_chain(st, 2, mc2)
                emit_group(st, 1, 2)
                masks_fin(st, 2, mc2)
                emit_group(st, 1, 3)
                emit_group(st, 1, 4)
                # phase C: step 2 with next-set prep interleaved
                nxt = sets[i + 1] if i + 1 < len(sets) else None
                stn = prep_A(nxt) if nxt else None
                emit_group(st, 2, 0)
                if stn:
                    prep_B(stn)
                emit_group(st, 2, 1)
                if stn:
                    mc1 = masks_rolls(stn, 1)
                    masks_cmps(stn, 1, mc1)
                emit_group(st, 2, 2)
                if stn:
                    masks_chain(stn, 1, mc1)
                    masks_fin(stn, 1, mc1)
                emit_group(st, 2, 3)
                if stn:
                    exact1(stn, mc1)
                    stn.w1b = w1pool.tile([128, C * W], F16, tag="w1b")
                emit_group(st, 2, 4)
                st = stn

    nc.compile()
    return nc


def _shift_mats():
    su = np.zeros((128, 128), np.float32)   # out[m] = in[m-1]
    sdn = np.zeros((128, 128), np.float32)  # out[m] = in[m+1]
    for m in range(128):
        if m >= 1:
            su[m - 1, m] = 1.0
        if m <= 126:
            sdn[m + 1, m] = 1.0
    sum_ = np.zeros((M_NP, M_NP), np.float32)
    sdm = np.zeros((M_NP, M_NP), np.float32)
    for base in (0, M_B1):
        for m in range(20):
            if m >= 1:
                sum_[base + m - 1, base + m] = 1.0
            if m <= 18:
                sdm[base + m + 1, base + m] = 1.0
    return su, sdn, sum_, sdm


_NC_CACHE = {}


def kernel(world, rand_movement=None, rand_interact=None, rand_element=None,
           **_ignored):
    world = np.ascontiguousarray(world, dtype=np.float32)
    assert world.shape == (B, C, H, W), world.shape
    if "nc" not in _NC_CACHE:
        _NC_CACHE["nc"] = build_kernel()
    nc = _NC_CACHE["nc"]
    su, sdn, sum_, sdm = _shift_mats()
    i16 = np.eye(128, dtype=np.float16)
    in_maps = []
    for core in range(N_CORES):
        shard = world[core * B_PER_CORE : (core + 1) * B_PER_CORE]
        in_maps.append({
            "world": np.ascontiguousarray(shard),
            "su32": su, "sd32": sdn,
            "su16": su.astype(np.float16), "sd16": sdn.astype(np.float16),
            "i16": i16,
            "sum32": sum_, "sdm32": sdm,
            "sum16": sum_.astype(np.float16),
            "sdm16": sdm.astype(np.float16),
        })
    res = run_bass_kernel_spmd(nc, in_maps, list(range(N_CORES)),
                               trace=_NC_CACHE.get("trace", False))
    _NC_CACHE["last_result"] = res
    out = np.concatenate([r["out16"] for r in res.results], axis=0)
    return out.astype(np.float32)


if __name__ == "__main__":
    rng = np.random.default_rng(0)
    w = rng.standard_normal((B, C, H, W)).astype(np.float32)
    w[:, 0] = rng.integers(0, 10, (B, H, W)).astype(np.float32)
    out = kernel(w)
    print("ran:", out.shape, out.dtype)


# revision 6
# speedup vs baseline: 1.0406x; 1.0399x over previous
"""Trainium2 Bass kernel for BehaviorLemming, v3.

Two fused stencil steps, data-parallel over batch (B=16 / 8 cores).
Layout: H rows in partitions, (channel, W) in free dim, 5 groups of 4ch.

v3 vs baseline:
- World movement in fp16: products P=a*w, Q=b*w, R=m0*w as fp16 tiles;
  row shifts as fp16 matmuls (1 cyc/row vs fp32's 4). The stay term R
  rides a third identity-matmul chain into PSUM, so the copy_predicated
  pass disappears; PSUM = su@Q + sd@P + I@R is the complete output.
- Exact fp32 side-path for step-1 density (ch1) only: step-2 mask
  comparisons must see bit-exact step-1 densities. Final outputs
  tolerate fp16 rounding (gate 2e-2, fp16 gives ~5e-4).
- Mask row-shifts (b = roll(a,-1), dA2 = roll(d2,+1)) as tiny matmuls;
  density-above (dA1) loaded straight from HBM at a row offset; mask
  W-rolls folded into shifted free-axis APs of the compare ops.
- Stores in fp16 (host converts): ~half the store traffic.
- P and R emitted as one double-wide DVE op over a packed [a16|m016]
  mask pair; Q split 2ch Pool / 2ch DVE; conversions + PSUM
  evacuations on Act. Deep software pipeline: per-engine queues run in
  emission order, so mask phases and next-set prep are interleaved
  between product groups that hide their latency.
"""

import numpy as np

import concourse.bacc as bacc
import concourse.mybir as mybir
import concourse.tile as tile
from concourse.bass_utils import run_bass_kernel_spmd

B, C, H, W = 16, 20, 512, 512
N_CORES = 8
B_PER_CORE = B // N_CORES
ELEM_ID = 3.0
F32 = mybir.dt.float32
F16 = mybir.dt.float16
NCH = 4
NGRP = C // NCH
MAIN_OUT = 124
M_B1 = 32              # partition offset of batch-1 block in the merged set
M_NP = 52

# knob per (step, group): where the Q product runs.
# "pool" = all 4ch on Pool, "split" = 2ch Pool + 2ch DVE, "dve" = all DVE
QMODE = {}
for _s in (1, 2):
    for _g in range(5):
        QMODE[(_s, _g)] = "split"
QMODE[(1, 4)] = "pool"
QMODE[(2, 4)] = "pool"
QMODE[(1, 0)] = "pool"
# which groups' conv (fp32->fp16 world copy) run on Act (rest DVE)
CONV_ACT = {0, 1, 2, 3, 4}


def _load_rows(nc, dst_tile, src_ap, row_start, n_rows, p0=0, nch=NCH):
    """Load n_rows (mod H, split at wrap) of src [nch,H,W] into dst
    partitions [p0, p0+n_rows), free dim = (c, w)."""
    s = row_start % H
    remaining = n_rows
    while remaining > 0:
        n = min(remaining, H - s)
        src = src_ap[:, s : s + n, :].rearrange("c h w -> h c w")
        nc.sync.dma_start(out=dst_tile[p0 : p0 + n, :].rearrange(
            "h (c w) -> h c w", c=nch), in_=src)
        p0 += n
        s = (s + n) % H
        remaining -= n


def _cmp_rolled(nc, al, out, rolled_src, base, shift_w, op):
    """out = op(roll(rolled_src, shift_w, W), base), via shifted free-axis
    APs: no materialized roll. Two pieces (bulk + 1-col wrap)."""
    if shift_w == 1:
        nc.vector.tensor_tensor(out=out[:, 1:W], in0=rolled_src[:, 0 : W - 1],
                                in1=base[:, 1:W], op=op)
        nc.vector.tensor_tensor(out=out[:, 0:1], in0=rolled_src[:, W - 1 : W],
                                in1=base[:, 0:1], op=op)
    else:
        nc.vector.tensor_tensor(out=out[:, 0 : W - 1], in0=rolled_src[:, 1:W],
                                in1=base[:, 0 : W - 1], op=op)
        nc.vector.tensor_tensor(out=out[:, W - 1 : W], in0=rolled_src[:, 0:1],
                                in1=base[:, W - 1 : W], op=op)


class SetCtx:
    """Per-set emission state."""

    def __init__(self, sd):
        self.sd = sd
        self.wb = None        # [np,10240] f16 world
        self.g0 = None        # [np,2048] f32 (ch0..3) for masks + exact
        self.dA1 = None       # [np,512] f32 density rolled +1 (HBM load)
        self.w1b = None       # [np,10240] f16 step-1 world
        self.w1d = None       # [np,512] f32 exact step-1 density
        self.m1 = None        # (a16, b16, m016) step-1
        self.m2 = None


def build_kernel():
    nc = bacc.Bacc("TRN2", target_bir_lowering=False, debug=False,
                   num_devices=N_CORES)
    wd = nc.dram_tensor("world", [B_PER_CORE, C, H, W], F32,
                        kind="ExternalInput").ap()
    su32_d = nc.dram_tensor("su32", [128, 128], F32, kind="ExternalInput").ap()
    sd32_d = nc.dram_tensor("sd32", [128, 128], F32, kind="ExternalInput").ap()
    su16_d = nc.dram_tensor("su16", [128, 128], F16, kind="ExternalInput").ap()
    sd16_d = nc.dram_tensor("sd16", [128, 128], F16, kind="ExternalInput").ap()
    i16_d = nc.dram_tensor("i16", [128, 128], F16, kind="ExternalInput").ap()
    sum32_d = nc.dram_tensor("sum32", [M_NP, M_NP], F32,
                             kind="ExternalInput").ap()
    sdm32_d = nc.dram_tensor("sdm32", [M_NP, M_NP], F32,
                             kind="ExternalInput").ap()
    sum16_d = nc.dram_tensor("sum16", [M_NP, M_NP], F16,
                             kind="ExternalInput").ap()
    sdm16_d = nc.dram_tensor("sdm16", [M_NP, M_NP], F16,
                             kind="ExternalInput").ap()
    od = nc.dram_tensor("out16", [B_PER_CORE, C, H, W], F16,
                        kind="ExternalOutput").ap()

    al = mybir.AluOpType

    with tile.TileContext(nc) as tc:
        with (
            tc.tile_pool(name="const", bufs=1) as cpool,
            tc.tile_pool(name="stg", bufs=2) as stgpool,
            tc.tile_pool(name="g0p", bufs=2) as g0pool,
            tc.tile_pool(name="wbp", bufs=2) as wbpool,
            tc.tile_pool(name="w1p", bufs=2) as w1pool,
            tc.tile_pool(name="mkp", bufs=2) as mkpool,
            tc.tile_pool(name="pqr", bufs=3) as pqrpool,
            tc.tile_pool(name="ogp", bufs=2) as ogpool,
            tc.tile_pool(name="pmain", bufs=4, space="PSUM") as pmain,
        ):
            su32 = cpool.tile([128, 128], F32)
            sd32 = cpool.tile([128, 128], F32)
            su16 = cpool.tile([128, 128], F16)
            sd16 = cpool.tile([128, 128], F16)
            i16 = cpool.tile([128, 128], F16)
            sum32 = cpool.tile([M_NP, M_NP], F32)
            sdm32 = cpool.tile([M_NP, M_NP], F32)
            sum16 = cpool.tile([M_NP, M_NP], F16)
            sdm16 = cpool.tile([M_NP, M_NP], F16)
            def load_consts():
                for t, d in ((su32, su32_d), (sd32, sd32_d), (su16, su16_d),
                             (sd16, sd16_d), (i16, i16_d), (sum32, sum32_d),
                             (sdm32, sdm32_d), (sum16, sum16_d),
                             (sdm16, sdm16_d)):
                    nc.sync.dma_start(out=t[:], in_=d)
            ones16 = cpool.tile([128, W], F16)
            nc.gpsimd.memset(ones16[:], 1.0)

            def masks_rolls(st, step):
                """Phase 1: dA2 = roll(d2,+1,H) via fp32 matmul (step 2)."""
                sd = st.sd
                np_ = sd["np"]
                if step == 1:
                    d = st.g0[0:np_, W : 2 * W]
                    dA = st.dA1[0:np_, :]
                    return {"d": d, "dA": dA}
                d = st.w1d[0:np_, :]
                psx = pmain.tile([np_, 2 * W], F32, tag="ps")
                nc.tensor.matmul(out=psx[:, 0:W], lhsT=sd["su32"], rhs=d,
                                 start=True, stop=True)
                dAt = mkpool.tile([np_, W], F32, tag="dA2")
                nc.scalar.copy(dAt[:], psx[:, 0:W])
                return {"d": d, "dA": dAt[:], "psx": psx}

            def masks_cmps(st, step, mc):
                """Phase 2: density comparisons via shifted free-axis APs."""
                np_ = st.sd["np"]
                shift_w = 1 if step == 1 else -1
                d, dA = mc["d"], mc["dA"]
                c1 = mkpool.tile([np_, W], F16, tag="c1")
                c2 = mkpool.tile([np_, W], F16, tag="c2")
                c3 = mkpool.tile([np_, W], F16, tag="c3")
                _cmp_rolled(nc, al, c1, d, d, shift_w, al.is_ge)
                nc.vector.tensor_tensor(out=c2[:], in0=dA, in1=d, op=al.is_lt)
                _cmp_rolled(nc, al, c3, dA, d, shift_w, al.is_lt)
                mc.update(c1=c1, c2=c2, c3=c3)

            def masks_chain(st, step, mc):
                """Phase 3: AND-tree -> a16; b16 = roll(a,-1,H) via matmul."""
                sd = st.sd
                np_ = sd["np"]
                e = st.g0[0:np_, 0:W] if step == 1 else st.w1b[0:np_, 0:W]
                e3c3 = mkpool.tile([np_, W], F16, tag="e3")
                nc.vector.scalar_tensor_tensor(out=e3c3[:], in0=e,
                                               scalar=ELEM_ID,
                                               in1=mc["c3"][:],
                                               op0=al.is_equal,
                                               op1=al.logical_and)
                c12 = mkpool.tile([np_, W], F16, tag="c12")
                nc.vector.tensor_tensor(out=c12[:], in0=mc["c1"][:],
                                        in1=mc["c2"][:], op=al.logical_and)
                mp = mkpool.tile([np_, 2 * W], F16, tag="mp")
                a16 = mp[:, 0:W]
                nc.vector.tensor_tensor(out=a16, in0=c12[:], in1=e3c3[:],
                                        op=al.logical_and)
                # b16[m] = a16[m+1]; sd16 zeroes the boundary rows natively
                psx = mc.get("psx")
                if psx is None:
                    psx = pmain.tile([np_, 2 * W], F32, tag="ps")
                    mc["psx"] = psx
                nc.tensor.matmul(out=psx[:, W : 2 * W], lhsT=sd["sd16"],
                                 rhs=a16, start=True, stop=True)
                b16 = mkpool.tile([np_, W], F16, tag="b16")
                nc.scalar.copy(b16[:], psx[:, W : 2 * W])
                mc.update(a16=a16, b16=b16, mp=mp)

            def masks_fin(st, step, mc):
                """Phase 4: r16/m016 (after the b16 DMA has had time)."""
                np_ = st.sd["np"]
                a16, b16 = mc["a16"], mc["b16"]
                r16 = mkpool.tile([np_, W], F16, tag="r16")
                nc.vector.tensor_tensor(out=r16[:], in0=a16, in1=b16[:],
                                        op=al.logical_or)
                m016 = mc["mp"][:, W : 2 * W]
                # r < 1 == (r == 0) for 0/1 masks; all-f16 operands -> 2x DVE
                nc.vector.tensor_tensor(out=m016, in0=r16[:],
                                        in1=ones16[0:np_, :], op=al.is_lt)
                if step == 1:
                    st.m1 = (mc["mp"], b16)
                else:
                    st.m2 = (mc["mp"], b16)

            def exact1(st, mc):
                """Exact fp32 density path (feeds step-2 comparisons)."""
                sd = st.sd
                np_ = sd["np"]
                d = st.g0[0:np_, W : 2 * W]
                mp, b16 = st.m1
                a16 = mp[:, 0:W]
                m016 = mp[:, W : 2 * W]
                P0 = mkpool.tile([np_, W], F32, tag="P0")
                Q0 = mkpool.tile([np_, W], F32, tag="Q0")
                R0 = mkpool.tile([np_, W], F32, tag="R0")
                nc.vector.tensor_tensor(out=P0[:], in0=a16, in1=d,
                                        op=al.mult)
                nc.vector.tensor_tensor(out=Q0[:], in0=b16[:], in1=d,
                                        op=al.mult)
                nc.vector.tensor_tensor(out=R0[:], in0=m016, in1=d,
                                        op=al.mult)
                psd = pmain.tile([np_, 2 * W], F32, tag="ps")
                nc.tensor.matmul(out=psd[:, 0:W], lhsT=sd["su32"],
                                 rhs=Q0[:], start=True, stop=False)
                nc.tensor.matmul(out=psd[:, 0:W], lhsT=sd["sd32"],
                                 rhs=P0[:], start=False, stop=True)
                w1d = w1pool.tile([np_, W], F32, tag="w1d")
                nc.vector.tensor_tensor(out=w1d[:], in0=psd[:, 0:W],
                                        in1=R0[:], op=al.add)
                st.w1d = w1d

            def emit_group(st, step, g):
                """One fp16 stencil group-step: products, matmuls, evac."""
                sd = st.sd
                np_ = sd["np"]
                mp, b16 = st.m1 if step == 1 else st.m2
                src = (st.wb if step == 1 else st.w1b)[
                    0:np_, g * NCH * W : (g + 1) * NCH * W]
                src_v = src.rearrange("p (c w) -> p c w", c=NCH)
                b_b = b16[:].unsqueeze(1).broadcast_to([np_, NCH, W])
                fd = NCH * W
                mode = QMODE[(step, g)]
                if sd.get("last") and step == 2 and g >= 3:
                    mode = "dve"    # shorten the drain tail
                # Q first: the su-chain consumes it before P/R are needed
                Q = pqrpool.tile([np_, fd], F16, tag="Q")
                Qv = Q[:].rearrange("p (c w) -> p c w", c=NCH)
                h = NCH // 2
                if mode == "split":
                    nc.gpsimd.tensor_tensor(
                        out=Qv[:, 0:h], in0=b_b[:, 0:h], in1=src_v[:, 0:h],
                        op=al.mult)
                    nc.vector.tensor_tensor(
                        out=Qv[:, h:NCH], in0=b_b[:, h:NCH],
                        in1=src_v[:, h:NCH], op=al.mult)
                    halves = ((2, 3), (0, 1))   # DVE-made half first
                elif mode == "pool":
                    # two half-ops so the first half's matmuls start sooner
                    nc.gpsimd.tensor_tensor(
                        out=Qv[:, 0:h], in0=b_b[:, 0:h], in1=src_v[:, 0:h],
                        op=al.mult)
                    nc.gpsimd.tensor_tensor(
                        out=Qv[:, h:NCH], in0=b_b[:, h:NCH],
                        in1=src_v[:, h:NCH], op=al.mult)
                    halves = ((0, 1), (2, 3))
                else:
                    nc.vector.tensor_tensor(out=Qv, in0=b_b, in1=src_v,
                                            op=al.mult)
                    halves = ((0, 1), (2, 3))
                # P and R as ONE double-wide DVE op: out [np, 2, NCH, W],
                # masks [a16 | m016] broadcast over channels, src broadcast
                # over the P/R axis.
                PR = pqrpool.tile([np_, 2 * fd], F16, tag="PR")
                PRv = PR[:].rearrange("p (k c w) -> p k c w", k=2, c=NCH)
                mp_b = mp.rearrange("p (k w) -> p k w", k=2).unsqueeze(
                    2).broadcast_to([np_, 2, NCH, W])
                src_b = src_v.unsqueeze(1).broadcast_to([np_, 2, NCH, W])
                nc.vector.tensor_tensor(out=PRv, in0=mp_b, in1=src_b,
                                        op=al.mult)
                P = PR[:, 0:fd]
                R = PR[:, fd : 2 * fd]
                if step == 1:
                    og = None
                    dst = st.w1b[0:np_, g * fd : (g + 1) * fd]
                else:
                    og = ogpool.tile([np_, fd], F16, tag="og")
                    dst = og[0:np_, :]
                hw = 2 * W
                for (c0, c1) in halves:
                    ps = pmain.tile([np_, hw], F32, tag="ps")
                    for c in (c0, c1):
                        r = slice((c - c0) * W, (c - c0 + 1) * W)
                        nc.tensor.matmul(out=ps[:, r], lhsT=sd["su16"],
                                         rhs=Q[:, c * W : (c + 1) * W],
                                         start=True, stop=False)
                        nc.tensor.matmul(out=ps[:, r], lhsT=sd["sd16"],
                                         rhs=P[:, c * W : (c + 1) * W],
                                         start=False, stop=False)
                        nc.tensor.matmul(out=ps[:, r], lhsT=sd["i16"],
                                         rhs=R[:, c * W : (c + 1) * W],
                                         start=False, stop=True)
                    nc.scalar.copy(dst[:, c0 * W : (c1 + 1) * W], ps[:])
                if og is not None:
                    sd["store"](og, g)

            def prep_A(sd):
                """Next-set loads for g0 + dA1, conv g0."""
                st = SetCtx(sd)
                np_ = sd["np"]
                st.g0 = g0pool.tile([128, NCH * W], F32, tag="g0")
                sd["load"](st.g0, 0)
                st.dA1 = g0pool.tile([128, W], F32, tag="dA1")
                sd["load_dA1"](st.dA1)
                st.wb = wbpool.tile([128, C * W], F16, tag="wb")
                nc.scalar.copy(st.wb[0:np_, 0 : NCH * W], st.g0[0:np_, :])
                return st

            def prep_B(st, mix_dve=False):
                """Remaining group loads + conversions. mix_dve spreads
                conversions over DVE too (startup, when DVE is idle)."""
                np_ = st.sd["np"]
                for g in range(1, NGRP):
                    stg = stgpool.tile([128, NCH * W], F32, tag="stg")
                    st.sd["load"](stg, g)
                    dst = st.wb[0:np_, g * NCH * W : (g + 1) * NCH * W]
                    if (g in CONV_ACT) and not (mix_dve and g % 2):
                        nc.scalar.copy(dst, stg[0:np_, :])
                    else:
                        nc.vector.tensor_copy(dst, stg[0:np_, :])

            def make_main_set(bi, si):
                r_out = si * MAIN_OUT

                def load(t, g):
                    _load_rows(nc, t, wd[bi, g * NCH : (g + 1) * NCH],
                               r_out - 2, 128)

                def load_dA1(t):
                    _load_rows(nc, t, wd[bi, 1:2], r_out - 3, 128, nch=1)

                def store(og, g):
                    dst = od[bi, g * NCH : (g + 1) * NCH,
                             r_out : r_out + MAIN_OUT, :]
                    nc.scalar.dma_start(
                        out=dst.rearrange("c h w -> h c w"),
                        in_=og[2 : 2 + MAIN_OUT, :].rearrange(
                            "h (c w) -> h c w", c=NCH))

                return {"np": 128, "su32": su32[:], "sd32": sd32[:],
                        "su16": su16[:], "sd16": sd16[:],
                        "i16": i16[:], "blocks": [(0, 128)],
                        "load": load, "load_dA1": load_dA1, "store": store}

            def make_merged_set():
                r_out = 4 * MAIN_OUT
                n_out = H - r_out        # 16
                blocks = [(0, n_out + 4), (M_B1, n_out + 4)]

                def load(t, g):
                    nc.gpsimd.memset(t[0:64, :], 0.0)
                    for bi, p0 in ((0, 0), (1, M_B1)):
                        _load_rows(nc, t, wd[bi, g * NCH : (g + 1) * NCH],
                                   r_out - 2, n_out + 4, p0=p0)

                def load_dA1(t):
                    nc.gpsimd.memset(t[0:64, :], 0.0)
                    for bi, p0 in ((0, 0), (1, M_B1)):
                        _load_rows(nc, t, wd[bi, 1:2], r_out - 3, n_out + 4,
                                   p0=p0, nch=1)

                def store(og, g):
                    for bi, p0 in ((0, 2), (1, M_B1 + 2)):
                        dst = od[bi, g * NCH : (g + 1) * NCH,
                                 r_out : r_out + n_out, :]
                        nc.scalar.dma_start(
                            out=dst.rearrange("c h w -> h c w"),
                            in_=og[p0 : p0 + n_out, :].rearrange(
                                "h (c w) -> h c w", c=NCH))

                return {"np": M_NP, "su32": sum32[:], "sd32": sdm32[:],
                        "su16": sum16[:], "sd16": sdm16[:],
                        "i16": i16[0:M_NP, 0:M_NP], "blocks": blocks,
                        "last": True,
                        "load": load, "load_dA1": load_dA1, "store": store}

            sets = [make_main_set(bi, si)
                    for bi in range(B_PER_CORE) for si in range(4)]
            sets.append(make_merged_set())

            # deep software pipeline. Per-engine queues run in emission
            # order, so long-latency chains (partition-shift DMAs, evac-
            # dependent compares) are split into phases and interleaved
            # between bulk product groups that hide their latency.
            st = prep_A(sets[0])
            prep_B(st, mix_dve=True)
            load_consts()
            mc1 = masks_rolls(st, 1)
            masks_cmps(st, 1, mc1)
            masks_chain(st, 1, mc1)
            masks_fin(st, 1, mc1)
            exact1(st, mc1)
            st.w1b = w1pool.tile([128, C * W], F16, tag="w1b")
            for i in range(len(sets)):
                # phase B: step 1 with step-2 mask phases interleaved
                emit_group(st, 1, 0)
                mc2 = masks_rolls(st, 2)
                masks_cmps(st, 2, mc2)
                emit_group(st, 1, 1)
                masks_chain(st, 2, mc2)
                emit_group(st, 1, 2)
                masks_fin(st, 2, mc2)
                emit_group(st, 1, 3)
                emit_group(st, 1, 4)
                # phase C: step 2 with next-set prep interleaved
                nxt = sets[i + 1] if i + 1 < len(sets) else None
                stn = prep_A(nxt) if nxt else None
                emit_group(st, 2, 0)
                if stn:
                    prep_B(stn)
                emit_group(st, 2, 1)
                if stn:
                    mc1 = masks_rolls(stn, 1)
                    masks_cmps(stn, 1, mc1)
                emit_group(st, 2, 2)
                if stn:
                    masks_chain(stn, 1, mc1)
                    masks_fin(stn, 1, mc1)
                emit_group(st, 2, 3)
                if stn:
                    exact1(stn, mc1)
                    stn.w1b = w1pool.tile([128, C * W], F16, tag="w1b")
                emit_group(st, 2, 4)
                st = stn

    nc.compile()
    return nc


def _shift_mats():
    su = np.zeros((128, 128), np.float32)   # out[m] = in[m-1]
    sdn = np.zeros((128, 128), np.float32)  # out[m] = in[m+1]
    for m in range(128):
        if m >= 1:
            su[m - 1, m] = 1.0
        if m <= 126:
            sdn[m + 1, m] = 1.0
    sum_ = np.zeros((M_NP, M_NP), np.float32)
    sdm = np.zeros((M_NP, M_NP), np.float32)
    for base in (0, M_B1):
        for m in range(20):
            if m >= 1:
                sum_[base + m - 1, base + m] = 1.0
            if m <= 18:
                sdm[base + m + 1, base + m] = 1.0
    return su, sdn, sum_, sdm


_NC_CACHE = {}


def kernel(world, rand_movement=None, rand_interact=None, rand_element=None,
           **_ignored):
    world = np.ascontiguousarray(world, dtype=np.float32)
    assert world.shape == (B, C, H, W), world.shape
    if "nc" not in _NC_CACHE:
        _NC_CACHE["nc"] = build_kernel()
    nc = _NC_CACHE["nc"]
    su, sdn, sum_, sdm = _shift_mats()
    i16 = np.eye(128, dtype=np.float16)
    in_maps = []
    for core in range(N_CORES):
        shard = world[core * B_PER_CORE : (core + 1) * B_PER_CORE]
        in_maps.append({
            "world": np.ascontiguousarray(shard),
            "su32": su, "sd32": sdn,
            "su16": su.astype(np.float16), "sd16": sdn.astype(np.float16),
            "i16": i16,
            "sum32": sum_, "sdm32": sdm,
            "sum16": sum_.astype(np.float16),
            "sdm16": sdm.astype(np.float16),
        })
    res = run_bass_kernel_spmd(nc, in_maps, list(range(N_CORES)),
                               trace=_NC_CACHE.get("trace", False))
    _NC_CACHE["last_result"] = res
    out = np.concatenate([r["out16"] for r in res.results], axis=0)
    return out.astype(np.float32)


if __name__ == "__main__":
    rng = np.random.default_rng(0)
    w = rng.standard_normal((B, C, H, W)).astype(np.float32)
    w[:, 0] = rng.integers(0, 10, (B, H, W)).astype(np.float32)
    out = kernel(w)
    print("ran:", out.shape, out.dtype)


# revision 8
# speedup vs baseline: 1.0524x; 1.0114x over previous
"""Trainium2 Bass kernel for BehaviorLemming, v3.

Two fused stencil steps, data-parallel over batch (B=16 / 8 cores).
Layout: H rows in partitions, (channel, W) in free dim, 5 groups of 4ch.

v3 vs baseline:
- World movement in fp16: products P=a*w, Q=b*w, R=m0*w as fp16 tiles;
  row shifts as fp16 matmuls (1 cyc/row vs fp32's 4). The stay term R
  rides a third identity-matmul chain into PSUM, so the copy_predicated
  pass disappears; PSUM = su@Q + sd@P + I@R is the complete output.
- Exact fp32 side-path for step-1 density (ch1) only: step-2 mask
  comparisons must see bit-exact step-1 densities. Final outputs
  tolerate fp16 rounding (gate 2e-2, fp16 gives ~5e-4).
- Mask row-shifts (b = roll(a,-1), dA2 = roll(d2,+1)) as tiny matmuls;
  density-above (dA1) loaded straight from HBM at a row offset; mask
  W-rolls folded into shifted free-axis APs of the compare ops.
- Stores in fp16 (host converts): ~half the store traffic.
- P and R emitted as one double-wide DVE op over a packed [a16|m016]
  mask pair; Q split 2ch Pool / 2ch DVE; conversions + PSUM
  evacuations on Act. Deep software pipeline: per-engine queues run in
  emission order, so mask phases and next-set prep are interleaved
  between product groups that hide their latency.
"""

import numpy as np

import concourse.bacc as bacc
import concourse.mybir as mybir
import concourse.tile as tile
from concourse.bass_utils import run_bass_kernel_spmd

B, C, H, W = 16, 20, 512, 512
N_CORES = 8
B_PER_CORE = B // N_CORES
ELEM_ID = 3.0
F32 = mybir.dt.float32
F16 = mybir.dt.float16
NCH = 4
NGRP = C // NCH
MAIN_OUT = 124
M_B1 = 32              # partition offset of batch-1 block in the merged set
M_NP = 52

# knob per (step, group): where the Q product runs.
# "pool" = all 4ch on Pool, "split" = 2ch Pool + 2ch DVE, "dve" = all DVE
QMODE = {}
for _s in (1, 2):
    for _g in range(5):
        QMODE[(_s, _g)] = "split"
QMODE[(1, 4)] = "pool"
QMODE[(2, 4)] = "pool"
QMODE[(1, 0)] = "pool"
# which groups' conv (fp32->fp16 world copy) run on Act (rest DVE)
CONV_ACT = {0, 1, 2, 3, 4}


def _load_rows(nc, dst_tile, src_ap, row_start, n_rows, p0=0, nch=NCH):
    """Load n_rows (mod H, split at wrap) of src [nch,H,W] into dst
    partitions [p0, p0+n_rows), free dim = (c, w)."""
    s = row_start % H
    remaining = n_rows
    while remaining > 0:
        n = min(remaining, H - s)
        src = src_ap[:, s : s + n, :].rearrange("c h w -> h c w")
        nc.sync.dma_start(out=dst_tile[p0 : p0 + n, :].rearrange(
            "h (c w) -> h c w", c=nch), in_=src)
        p0 += n
        s = (s + n) % H
        remaining -= n


def _cmp_rolled(nc, al, out, rolled_src, base, shift_w, op):
    """out = op(roll(rolled_src, shift_w, W), base), via shifted free-axis
    APs: no materialized roll. Two pieces (bulk + 1-col wrap)."""
    if shift_w == 1:
        nc.vector.tensor_tensor(out=out[:, 1:W], in0=rolled_src[:, 0 : W - 1],
                                in1=base[:, 1:W], op=op)
        nc.vector.tensor_tensor(out=out[:, 0:1], in0=rolled_src[:, W - 1 : W],
                                in1=base[:, 0:1], op=op)
    else:
        nc.vector.tensor_tensor(out=out[:, 0 : W - 1], in0=rolled_src[:, 1:W],
                                in1=base[:, 0 : W - 1], op=op)
        nc.vector.tensor_tensor(out=out[:, W - 1 : W], in0=rolled_src[:, 0:1],
                                in1=base[:, W - 1 : W], op=op)


class SetCtx:
    """Per-set emission state."""

    def __init__(self, sd):
        self.sd = sd
        self.wb = None        # [np,10240] f16 world
        self.g0 = None        # [np,2048] f32 (ch0..3) for masks + exact
        self.dA1 = None       # [np,512] f32 density rolled +1 (HBM load)
        self.w1b = None       # [np,10240] f16 step-1 world
        self.w1d = None       # [np,512] f32 exact step-1 density
        self.m1 = None        # (a16, b16, m016) step-1
        self.m2 = None


def build_kernel():
    nc = bacc.Bacc("TRN2", target_bir_lowering=False, debug=False,
                   num_devices=N_CORES)
    wd = nc.dram_tensor("world", [B_PER_CORE, C, H, W], F32,
                        kind="ExternalInput").ap()
    su32_d = nc.dram_tensor("su32", [128, 128], F32, kind="ExternalInput").ap()
    sd32_d = nc.dram_tensor("sd32", [128, 128], F32, kind="ExternalInput").ap()
    su16_d = nc.dram_tensor("su16", [128, 128], F16, kind="ExternalInput").ap()
    sd16_d = nc.dram_tensor("sd16", [128, 128], F16, kind="ExternalInput").ap()
    i16_d = nc.dram_tensor("i16", [128, 128], F16, kind="ExternalInput").ap()
    i32_d = nc.dram_tensor("i32", [128, 128], F32, kind="ExternalInput").ap()
    sum32_d = nc.dram_tensor("sum32", [M_NP, M_NP], F32,
                             kind="ExternalInput").ap()
    sdm32_d = nc.dram_tensor("sdm32", [M_NP, M_NP], F32,
                             kind="ExternalInput").ap()
    sum16_d = nc.dram_tensor("sum16", [M_NP, M_NP], F16,
                             kind="ExternalInput").ap()
    sdm16_d = nc.dram_tensor("sdm16", [M_NP, M_NP], F16,
                             kind="ExternalInput").ap()
    od = nc.dram_tensor("out16", [B_PER_CORE, C, H, W], F16,
                        kind="ExternalOutput").ap()

    al = mybir.AluOpType

    with tile.TileContext(nc) as tc:
        with (
            tc.tile_pool(name="const", bufs=1) as cpool,
            tc.tile_pool(name="stg", bufs=2) as stgpool,
            tc.tile_pool(name="g0p", bufs=2) as g0pool,
            tc.tile_pool(name="wbp", bufs=2) as wbpool,
            tc.tile_pool(name="w1p", bufs=2) as w1pool,
            tc.tile_pool(name="mkp", bufs=2) as mkpool,
            tc.tile_pool(name="pqr", bufs=3) as pqrpool,
            tc.tile_pool(name="ogp", bufs=2) as ogpool,
            tc.tile_pool(name="pmain", bufs=4, space="PSUM") as pmain,
        ):
            su32 = cpool.tile([128, 128], F32)
            sd32 = cpool.tile([128, 128], F32)
            su16 = cpool.tile([128, 128], F16)
            sd16 = cpool.tile([128, 128], F16)
            i16 = cpool.tile([128, 128], F16)
            i32 = cpool.tile([128, 128], F32)
            sum32 = cpool.tile([M_NP, M_NP], F32)
            sdm32 = cpool.tile([M_NP, M_NP], F32)
            sum16 = cpool.tile([M_NP, M_NP], F16)
            sdm16 = cpool.tile([M_NP, M_NP], F16)
            def load_consts():
                for t, d in ((su32, su32_d), (sd32, sd32_d), (su16, su16_d),
                             (sd16, sd16_d), (i16, i16_d), (i32, i32_d),
                             (sum32, sum32_d),
                             (sdm32, sdm32_d), (sum16, sum16_d),
                             (sdm16, sdm16_d)):
                    nc.sync.dma_start(out=t[:], in_=d)
            ones16 = cpool.tile([128, W], F16)
            nc.gpsimd.memset(ones16[:], 1.0)

            def masks_rolls(st, step):
                """Phase 1: dA2 = roll(d2,+1,H) via fp32 matmul (step 2)."""
                sd = st.sd
                np_ = sd["np"]
                if step == 1:
                    d = st.g0[0:np_, W : 2 * W]
                    dA = st.dA1[0:np_, :]
                    return {"d": d, "dA": dA}
                d = st.w1d[0:np_, :]
                psx = pmain.tile([np_, 2 * W], F32, tag="ps")
                nc.tensor.matmul(out=psx[:, 0:W], lhsT=sd["su32"], rhs=d,
                                 start=True, stop=True)
                dAt = mkpool.tile([np_, W], F32, tag="dA2")
                nc.scalar.copy(dAt[:], psx[:, 0:W])
                return {"d": d, "dA": dAt[:], "psx": psx}

            def masks_cmps(st, step, mc):
                """Phase 2: density comparisons via shifted free-axis APs."""
                np_ = st.sd["np"]
                shift_w = 1 if step == 1 else -1
                d, dA = mc["d"], mc["dA"]
                c1 = mkpool.tile([np_, W], F16, tag="c1")
                c2 = mkpool.tile([np_, W], F16, tag="c2")
                c3 = mkpool.tile([np_, W], F16, tag="c3")
                _cmp_rolled(nc, al, c1, d, d, shift_w, al.is_ge)
                nc.vector.tensor_tensor(out=c2[:], in0=dA, in1=d, op=al.is_lt)
                _cmp_rolled(nc, al, c3, dA, d, shift_w, al.is_lt)
                mc.update(c1=c1, c2=c2, c3=c3)

            def masks_chain(st, step, mc):
                """Phase 3: AND-tree -> a16; b16 = roll(a,-1,H) via matmul."""
                sd = st.sd
                np_ = sd["np"]
                e = st.g0[0:np_, 0:W] if step == 1 else st.w1b[0:np_, 0:W]
                e3c3 = mkpool.tile([np_, W], F16, tag="e3")
                nc.vector.scalar_tensor_tensor(out=e3c3[:], in0=e,
                                               scalar=ELEM_ID,
                                               in1=mc["c3"][:],
                                               op0=al.is_equal,
                                               op1=al.logical_and)
                c12 = mkpool.tile([np_, W], F16, tag="c12")
                nc.vector.tensor_tensor(out=c12[:], in0=mc["c1"][:],
                                        in1=mc["c2"][:], op=al.logical_and)
                mp = mkpool.tile([np_, 2 * W], F16, tag="mp")
                a16 = mp[:, 0:W]
                nc.vector.tensor_tensor(out=a16, in0=c12[:], in1=e3c3[:],
                                        op=al.logical_and)
                # b16[m] = a16[m+1]; sd16 zeroes the boundary rows natively
                psx = mc.get("psx")
                if psx is None:
                    psx = pmain.tile([np_, 2 * W], F32, tag="ps")
                    mc["psx"] = psx
                nc.tensor.matmul(out=psx[:, W : 2 * W], lhsT=sd["sd16"],
                                 rhs=a16, start=True, stop=True)
                b16 = mkpool.tile([np_, W], F16, tag="b16")
                nc.scalar.copy(b16[:], psx[:, W : 2 * W])
                mc.update(a16=a16, b16=b16, mp=mp)

            def masks_fin(st, step, mc):
                """Phase 4: r16/m016 (after the b16 DMA has had time)."""
                np_ = st.sd["np"]
                a16, b16 = mc["a16"], mc["b16"]
                r16 = mkpool.tile([np_, W], F16, tag="r16")
                nc.vector.tensor_tensor(out=r16[:], in0=a16, in1=b16[:],
                                        op=al.logical_or)
                m016 = mc["mp"][:, W : 2 * W]
                # r < 1 == (r == 0) for 0/1 masks; all-f16 operands -> 2x DVE
                nc.vector.tensor_tensor(out=m016, in0=r16[:],
                                        in1=ones16[0:np_, :], op=al.is_lt)
                if step == 1:
                    st.m1 = (mc["mp"], b16)
                else:
                    st.m2 = (mc["mp"], b16)

            def exact1(st, mc):
                """Exact fp32 density path (feeds step-2 comparisons)."""
                sd = st.sd
                np_ = sd["np"]
                d = st.g0[0:np_, W : 2 * W]
                mp, b16 = st.m1
                a16 = mp[:, 0:W]
                m016 = mp[:, W : 2 * W]
                P0 = mkpool.tile([np_, W], F32, tag="P0")
                Q0 = mkpool.tile([np_, W], F32, tag="Q0")
                R0 = mkpool.tile([np_, W], F32, tag="R0")
                nc.vector.tensor_tensor(out=P0[:], in0=a16, in1=d,
                                        op=al.mult)
                nc.vector.tensor_tensor(out=Q0[:], in0=b16[:], in1=d,
                                        op=al.mult)
                nc.vector.tensor_tensor(out=R0[:], in0=m016, in1=d,
                                        op=al.mult)
                psd = pmain.tile([np_, 2 * W], F32, tag="ps")
                nc.tensor.matmul(out=psd[:, 0:W], lhsT=sd["su32"],
                                 rhs=Q0[:], start=True, stop=False)
                nc.tensor.matmul(out=psd[:, 0:W], lhsT=sd["sd32"],
                                 rhs=P0[:], start=False, stop=True)
                w1d = w1pool.tile([np_, W], F32, tag="w1d")
                nc.vector.tensor_tensor(out=w1d[:], in0=psd[:, 0:W],
                                        in1=R0[:], op=al.add)
                st.w1d = w1d

            def emit_group(st, step, g):
                """One fp16 stencil group-step: products, matmuls, evac."""
                sd = st.sd
                np_ = sd["np"]
                mp, b16 = st.m1 if step == 1 else st.m2
                src = (st.wb if step == 1 else st.w1b)[
                    0:np_, g * NCH * W : (g + 1) * NCH * W]
                src_v = src.rearrange("p (c w) -> p c w", c=NCH)
                b_b = b16[:].unsqueeze(1).broadcast_to([np_, NCH, W])
                fd = NCH * W
                mode = QMODE[(step, g)]
                if sd.get("last") and step == 2 and g >= 3:
                    mode = "dve"    # shorten the drain tail
                # Q first: the su-chain consumes it before P/R are needed
                Q = pqrpool.tile([np_, fd], F16, tag="Q")
                Qv = Q[:].rearrange("p (c w) -> p c w", c=NCH)
                h = NCH // 2
                if mode in ("split", "rpool"):
                    nc.gpsimd.tensor_tensor(
                        out=Qv[:, 0:h], in0=b_b[:, 0:h], in1=src_v[:, 0:h],
                        op=al.mult)
                    nc.vector.tensor_tensor(
                        out=Qv[:, h:NCH], in0=b_b[:, h:NCH],
                        in1=src_v[:, h:NCH], op=al.mult)
                    halves = ((2, 3), (0, 1))   # DVE-made half first
                elif mode == "pool":
                    # two half-ops so the first half's matmuls start sooner
                    nc.gpsimd.tensor_tensor(
                        out=Qv[:, 0:h], in0=b_b[:, 0:h], in1=src_v[:, 0:h],
                        op=al.mult)
                    nc.gpsimd.tensor_tensor(
                        out=Qv[:, h:NCH], in0=b_b[:, h:NCH],
                        in1=src_v[:, h:NCH], op=al.mult)
                    halves = ((0, 1), (2, 3))
                else:
                    nc.vector.tensor_tensor(out=Qv, in0=b_b, in1=src_v,
                                            op=al.mult)
                    halves = ((0, 1), (2, 3))
                PR = pqrpool.tile([np_, 2 * fd], F16, tag="PR")
                if mode == "rpool":
                    # P on DVE; R fully on Pool (I-chain consumes R last,
                    # so Pool's latency is tolerable)
                    a_b = mp[:, 0:W].unsqueeze(1).broadcast_to(
                        [np_, NCH, W])
                    m_b = mp[:, W : 2 * W].unsqueeze(1).broadcast_to(
                        [np_, NCH, W])
                    nc.vector.tensor_tensor(
                        out=PR[:, 0:fd].rearrange("p (c w) -> p c w", c=NCH),
                        in0=a_b, in1=src_v, op=al.mult)
                    nc.gpsimd.tensor_tensor(
                        out=PR[:, fd : 2 * fd].rearrange(
                            "p (c w) -> p c w", c=NCH),
                        in0=m_b, in1=src_v, op=al.mult)
                else:
                    # P and R as ONE double-wide DVE op: out [np,2,NCH,W],
                    # masks [a16 | m016] broadcast over channels, src
                    # broadcast over the P/R axis.
                    PRv = PR[:].rearrange("p (k c w) -> p k c w", k=2, c=NCH)
                    mp_b = mp.rearrange("p (k w) -> p k w", k=2).unsqueeze(
                        2).broadcast_to([np_, 2, NCH, W])
                    src_b = src_v.unsqueeze(1).broadcast_to([np_, 2, NCH, W])
                    nc.vector.tensor_tensor(out=PRv, in0=mp_b, in1=src_b,
                                            op=al.mult)
                P = PR[:, 0:fd]
                R = PR[:, fd : 2 * fd]
                if step == 1:
                    og = None
                    dst = st.w1b[0:np_, g * fd : (g + 1) * fd]
                else:
                    og = ogpool.tile([np_, fd], F16, tag="og")
                    dst = og[0:np_, :]
                hw = 2 * W
                for (c0, c1) in halves:
                    ps = pmain.tile([np_, hw], F32, tag="ps")
                    for c in (c0, c1):
                        r = slice((c - c0) * W, (c - c0 + 1) * W)
                        nc.tensor.matmul(out=ps[:, r], lhsT=sd["su16"],
                                         rhs=Q[:, c * W : (c + 1) * W],
                                         start=True, stop=False)
                        nc.tensor.matmul(out=ps[:, r], lhsT=sd["sd16"],
                                         rhs=P[:, c * W : (c + 1) * W],
                                         start=False, stop=False)
                        nc.tensor.matmul(out=ps[:, r], lhsT=sd["i16"],
                                         rhs=R[:, c * W : (c + 1) * W],
                                         start=False, stop=True)
                    nc.scalar.copy(dst[:, c0 * W : (c1 + 1) * W], ps[:])
                if og is not None:
                    sd["store"](og, g)

            def prep_A(sd):
                """Next-set loads for g0 + dA1, conv g0."""
                st = SetCtx(sd)
                np_ = sd["np"]
                st.g0 = g0pool.tile([128, NCH * W], F32, tag="g0")
                sd["load"](st.g0, 0)
                st.dA1 = g0pool.tile([128, W], F32, tag="dA1")
                sd["load_dA1"](st.dA1)
                st.wb = wbpool.tile([128, C * W], F16, tag="wb")
                nc.scalar.copy(st.wb[0:np_, 0 : NCH * W], st.g0[0:np_, :])
                return st

            def prep_B(st, mix_dve=False):
                """Remaining group loads + conversions. mix_dve spreads
                conversions over DVE too (startup, when DVE is idle)."""
                np_ = st.sd["np"]
                for g in range(1, NGRP):
                    stg = stgpool.tile([128, NCH * W], F32, tag="stg")
                    st.sd["load"](stg, g)
                    dst = st.wb[0:np_, g * NCH * W : (g + 1) * NCH * W]
                    if (g in CONV_ACT) and not (mix_dve and g % 2):
                        nc.scalar.copy(dst, stg[0:np_, :])
                    else:
                        nc.vector.tensor_copy(dst, stg[0:np_, :])

            def make_main_set(bi, si):
                r_out = si * MAIN_OUT

                def load(t, g):
                    _load_rows(nc, t, wd[bi, g * NCH : (g + 1) * NCH],
                               r_out - 2, 128)

                def load_dA1(t):
                    _load_rows(nc, t, wd[bi, 1:2], r_out - 3, 128, nch=1)

                def store(og, g):
                    dst = od[bi, g * NCH : (g + 1) * NCH,
                             r_out : r_out + MAIN_OUT, :]
                    nc.scalar.dma_start(
                        out=dst.rearrange("c h w -> h c w"),
                        in_=og[2 : 2 + MAIN_OUT, :].rearrange(
                            "h (c w) -> h c w", c=NCH))

                return {"np": 128, "su32": su32[:], "sd32": sd32[:],
                        "su16": su16[:], "sd16": sd16[:],
                        "i16": i16[:], "i32": i32[:], "blocks": [(0, 128)],
                        "load": load, "load_dA1": load_dA1, "store": store}

            def make_merged_set():
                r_out = 4 * MAIN_OUT
                n_out = H - r_out        # 16
                blocks = [(0, n_out + 4), (M_B1, n_out + 4)]

                def load(t, g):
                    nc.gpsimd.memset(t[0:64, :], 0.0)
                    for bi, p0 in ((0, 0), (1, M_B1)):
                        _load_rows(nc, t, wd[bi, g * NCH : (g + 1) * NCH],
                                   r_out - 2, n_out + 4, p0=p0)

                def load_dA1(t):
                    nc.gpsimd.memset(t[0:64, :], 0.0)
                    for bi, p0 in ((0, 0), (1, M_B1)):
                        _load_rows(nc, t, wd[bi, 1:2], r_out - 3, n_out + 4,
                                   p0=p0, nch=1)

                def store(og, g):
                    for bi, p0 in ((0, 2), (1, M_B1 + 2)):
                        dst = od[bi, g * NCH : (g + 1) * NCH,
                                 r_out : r_out + n_out, :]
                        nc.scalar.dma_start(
                            out=dst.rearrange("c h w -> h c w"),
                            in_=og[p0 : p0 + n_out, :].rearrange(
                                "h (c w) -> h c w", c=NCH))

                return {"np": M_NP, "su32": sum32[:], "sd32": sdm32[:],
                        "su16": sum16[:], "sd16": sdm16[:],
                        "i16": i16[0:M_NP, 0:M_NP],
                        "i32": i32[0:M_NP, 0:M_NP], "blocks": blocks,
                        "last": True,
                        "load": load, "load_dA1": load_dA1, "store": store}

            sets = [make_main_set(bi, si)
                    for bi in range(B_PER_CORE) for si in range(4)]
            sets.append(make_merged_set())

            # deep software pipeline. Per-engine queues run in emission
            # order, so long-latency chains (partition-shift DMAs, evac-
            # dependent compares) are split into phases and interleaved
            # between bulk product groups that hide their latency.
            st = prep_A(sets[0])
            prep_B(st, mix_dve=True)
            load_consts()
            mc1 = masks_rolls(st, 1)
            masks_cmps(st, 1, mc1)
            masks_chain(st, 1, mc1)
            masks_fin(st, 1, mc1)
            exact1(st, mc1)
            st.w1b = w1pool.tile([128, C * W], F16, tag="w1b")
            for i in range(len(sets)):
                # phase B: step 1 with step-2 mask phases interleaved
                emit_group(st, 1, 0)
                mc2 = masks_rolls(st, 2)
                masks_cmps(st, 2, mc2)
                emit_group(st, 1, 1)
                masks_chain(st, 2, mc2)
                emit_group(st, 1, 2)
                masks_fin(st, 2, mc2)
                emit_group(st, 1, 3)
                emit_group(st, 1, 4)
                # phase C: step 2 with next-set prep interleaved
                nxt = sets[i + 1] if i + 1 < len(sets) else None
                stn = prep_A(nxt) if nxt else None
                emit_group(st, 2, 0)
                if stn:
                    prep_B(stn)
                emit_group(st, 2, 1)
                if stn:
                    mc1 = masks_rolls(stn, 1)
                    masks_cmps(stn, 1, mc1)
                emit_group(st, 2, 2)
                if stn:
                    masks_chain(stn, 1, mc1)
                    masks_fin(stn, 1, mc1)
                emit_group(st, 2, 3)
                if stn:
                    exact1(stn, mc1)
                    stn.w1b = w1pool.tile([128, C * W], F16, tag="w1b")
                emit_group(st, 2, 4)
                st = stn

    nc.compile()
    return nc


def _shift_mats():
    su = np.zeros((128, 128), np.float32)   # out[m] = in[m-1]
    sdn = np.zeros((128, 128), np.float32)  # out[m] = in[m+1]
    for m in range(128):
        if m >= 1:
            su[m - 1, m] = 1.0
        if m <= 126:
            sdn[m + 1, m] = 1.0
    sum_ = np.zeros((M_NP, M_NP), np.float32)
    sdm = np.zeros((M_NP, M_NP), np.float32)
    for base in (0, M_B1):
        for m in range(20):
            if m >= 1:
                sum_[base + m - 1, base + m] = 1.0
            if m <= 18:
                sdm[base + m + 1, base + m] = 1.0
    return su, sdn, sum_, sdm


_NC_CACHE = {}


def kernel(world, rand_movement=None, rand_interact=None, rand_element=None,
           **_ignored):
    world = np.ascontiguousarray(world, dtype=np.float32)
    assert world.shape == (B, C, H, W), world.shape
    if "nc" not in _NC_CACHE:
        _NC_CACHE["nc"] = build_kernel()
    nc = _NC_CACHE["nc"]
    su, sdn, sum_, sdm = _shift_mats()
    i16 = np.eye(128, dtype=np.float16)
    in_maps = []
    for core in range(N_CORES):
        shard = world[core * B_PER_CORE : (core + 1) * B_PER_CORE]
        in_maps.append({
            "world": np.ascontiguousarray(shard),
            "su32": su, "sd32": sdn,
            "su16": su.astype(np.float16), "sd16": sdn.astype(np.float16),
            "i16": i16, "i32": np.eye(128, dtype=np.float32),
            "sum32": sum_, "sdm32": sdm,
            "sum16": sum_.astype(np.float16),
            "sdm16": sdm.astype(np.float16),
        })
    res = run_bass_kernel_spmd(nc, in_maps, list(range(N_CORES)),
                               trace=_NC_CACHE.get("trace", False))
    _NC_CACHE["last_result"] = res
    out = np.concatenate([r["out16"] for r in res.results], axis=0)
    return out.astype(np.float32)


if __name__ == "__main__":
    rng = np.random.default_rng(0)
    w = rng.standard_normal((B, C, H, W)).astype(np.float32)
    w[:, 0] = rng.integers(0, 10, (B, H, W)).astype(np.float32)
    out = kernel(w)
    print("ran:", out.shape, out.dtype)
